# revision 13
# baseline (speedup 1.0000x reference)
import sys

if "/opt/trn_rl_repo" not in sys.path:
    sys.path.insert(0, "/opt/trn_rl_repo")

import numpy as np

B, S, V, D = 256, 512, 100, 64
NCORES = 8
R = B // NCORES  # rows per core

# bf16 const tile column layout [128, CB]
CB_W1R0 = 0
CB_W1R1 = 64
CB_B1 = 128
CB_W2 = 192
CB_SEL = 256
CB = 288

# f32 const tile column layout [128, CF]
CF_VIDX = 0
CF_B2 = 1
CF = 2

_CACHE = {}
LAST_RESULT = None


def _emit(ctx, nc, tc, idsb, cstb, cstf, out):
    from concourse import bass

    mybir = bass.mybir
    f32 = mybir.dt.float32
    bf16 = mybir.dt.bfloat16
    alu = mybir.AluOpType
    act = mybir.ActivationFunctionType

    consts_p = ctx.enter_context(tc.tile_pool(name="cst", bufs=1))
    oh_p = ctx.enter_context(tc.tile_pool(name="oh", bufs=3))
    h_p = ctx.enter_context(tc.tile_pool(name="h", bufs=3))
    mlp_p = ctx.enter_context(tc.tile_pool(name="mlp", bufs=3))
    tt_p = ctx.enter_context(tc.tile_pool(name="tt", bufs=3))
    ht_p = ctx.enter_context(tc.tile_pool(name="ht", bufs=3))
    tab_p = ctx.enter_context(tc.tile_pool(name="tab", bufs=3))
    gout_p = ctx.enter_context(tc.tile_pool(name="gout", bufs=3))
    ps_bc = ctx.enter_context(tc.tile_pool(name="ps_bc", bufs=3, space="PSUM"))
    ps_tab = ctx.enter_context(tc.tile_pool(name="ps_tab", bufs=2, space="PSUM"))
    ps_g = ctx.enter_context(tc.tile_pool(name="ps_g", bufs=3, space="PSUM"))

    sb_ids = consts_p.tile([R, 2 * S], bf16)
    nc.sync.dma_start(out=sb_ids, in_=idsb)
    cb = consts_p.tile([128, CB], bf16)
    for p in range(0, 128, 32):
        nc.scalar.dma_start(out=cb[p : p + 32, :], in_=cstb[p : p + 32, :])
    cf = consts_p.tile([128, CF], f32)
    for p in range(0, 128, 32):
        nc.scalar.dma_start(out=cf[p : p + 32, :], in_=cstf[p : p + 32, :])

    w1r0 = cb[0:112, CB_W1R0 : CB_W1R0 + D]
    w1r1 = cb[0:112, CB_W1R1 : CB_W1R1 + D]
    b1rep = cb[0:112, CB_B1 : CB_B1 + D]
    w2 = cb[0:D, CB_W2 : CB_W2 + D]
    sel = cb[0:R, CB_SEL : CB_SEL + R]
    vidx = cf[:, CF_VIDX : CF_VIDX + 1]
    b2c = cf[0:D, CF_B2 : CF_B2 + 1]

    for r in range(R):
        # replicate row r's ids (src | dst) to all 128 partitions
        lhsT = sel[:, r : r + 1].broadcast_to((R, 128))
        psb_s = ps_bc.tile([128, S], f32, tag="psb")
        psb_d = ps_bc.tile([128, S], f32, tag="psb")
        nc.tensor.matmul(out=psb_s, lhsT=lhsT, rhs=sb_ids[:, 0:S])
        nc.tensor.matmul(out=psb_d, lhsT=lhsT, rhs=sb_ids[:, S : 2 * S])

        # one-hot + histogram per side
        oh = oh_p.tile([128, 2 * S], bf16)
        h = h_p.tile([128, 2], f32)
        nc.vector.tensor_scalar(
            out=oh[:, 0:S], in0=psb_s, scalar1=vidx, scalar2=None,
            op0=alu.is_equal, op1=alu.add, accum_out=h[:, 0:1],
        )
        nc.vector.tensor_scalar(
            out=oh[:, S : 2 * S], in0=psb_d, scalar1=vidx,
            scalar2=None, op0=alu.is_equal, op1=alu.add, accum_out=h[:, 1:2],
        )
        # padding id 0 contributes zero features to the MLP
        nc.gpsimd.memset(h[0:1, 0:2], 0.0)

        # P = relu(h_s*W1[0] + h_d*W1[1] + b1); the whole [112,128] transpose
        # input is written (tmp parked in cols 64:128) so no pad is left
        # uninitialized. Rows 100:112 compute on h=0 and are never consumed.
        ttile = tt_p.tile([112, 128], bf16)
        nc.vector.scalar_tensor_tensor(
            out=ttile[:, D : 2 * D], in0=w1r0, scalar=h[0:112, 0:1], in1=b1rep,
            op0=alu.mult, op1=alu.add,
        )
        hpre = mlp_p.tile([112, D], bf16)
        nc.vector.scalar_tensor_tensor(
            out=hpre, in0=w1r1, scalar=h[0:112, 1:2], in1=ttile[:, D : 2 * D],
            op0=alu.mult, op1=alu.add,
        )
        nc.scalar.activation(out=ttile[:, 0:D], in_=hpre, func=act.Relu)

        # transpose [112,128] -> [128,112]; valid region hT = out[0:64, 0:100]
        htile = ht_p.tile([128, 112], bf16)
        nc.scalar.dma_start_transpose(out=htile, in_=ttile)

        # tab[v,f] = hrelu^T.T @ W2  (contraction over g=64)
        pst = ps_tab.tile([V, D], f32)
        nc.tensor.matmul(out=pst, lhsT=htile[0:D, 0:V], rhs=w2)
        tab = tab_p.tile([V, D], bf16)
        nc.scalar.copy(out=tab, in_=pst)

        # gather: out[f, s] = tab[ids[s], f] via one-hot matmul
        psg_s = ps_g.tile([D, S], f32, tag="psg")
        psg_d = ps_g.tile([D, S], f32, tag="psg")
        nc.tensor.matmul(out=psg_s, lhsT=tab, rhs=oh[0:V, 0:S])
        nc.tensor.matmul(out=psg_d, lhsT=tab, rhs=oh[0:V, S : 2 * S])

        # psum -> bf16 with b2 added as per-partition bias
        gout = gout_p.tile([D, 2 * S], bf16)
        nc.scalar.activation(
            out=gout[:, 0:S], in_=psg_s, func=act.Identity, bias=b2c,
        )
        nc.scalar.activation(
            out=gout[:, S : 2 * S], in_=psg_d, func=act.Identity,
            bias=b2c,
        )
        nc.sync.dma_start(
            out=out[:, r * 2 * S : (r + 1) * 2 * S], in_=gout
        )


def _build_module():
    from contextlib import ExitStack

    from concourse import bacc, bass, tile

    mybir = bass.mybir
    nc = bacc.Bacc(
        "TRN2", target_bir_lowering=False, debug=False, num_devices=NCORES
    )
    idsb = nc.dram_tensor(
        "idsb", [R, 2 * S], mybir.dt.bfloat16, kind="ExternalInput"
    ).ap()
    cstb = nc.dram_tensor(
        "cstb", [128, CB], mybir.dt.bfloat16, kind="ExternalInput"
    ).ap()
    cstf = nc.dram_tensor(
        "cstf", [128, CF], mybir.dt.float32, kind="ExternalInput"
    ).ap()
    out = nc.dram_tensor(
        "out", [D, R * 2 * S], mybir.dt.bfloat16, kind="ExternalOutput"
    ).ap()

    with tile.TileContext(nc) as tc:
        with ExitStack() as ctx:
            _emit(ctx, nc, tc, idsb, cstb, cstf, out)
    nc.finalize()
    return nc


def get_module():
    if "nc" not in _CACHE:
        _CACHE["nc"] = _build_module()
    return _CACHE["nc"]


def _build_cstb(W1, b1, W2):
    import ml_dtypes

    c = np.zeros((128, CB), np.float32)
    c[:, CB_W1R0 : CB_W1R0 + D] = W1[0]
    c[:, CB_W1R1 : CB_W1R1 + D] = W1[1]
    c[:, CB_B1 : CB_B1 + D] = b1
    c[0:D, CB_W2 : CB_W2 + D] = W2
    c[0:R, CB_SEL : CB_SEL + R] = np.eye(R, dtype=np.float32)
    return c.astype(ml_dtypes.bfloat16)


def _build_cstf(b2):
    c = np.zeros((128, CF), np.float32)
    c[:, CF_VIDX] = np.arange(128, dtype=np.float32)
    c[0:D, CF_B2] = b2
    return c


def kernel(**inputs):
    global LAST_RESULT
    import ml_dtypes

    from concourse import bass_utils

    src = np.asarray(inputs["src_neighbor_ids"])
    dst = np.asarray(inputs["dst_neighbor_ids"])
    W1 = np.asarray(inputs["W1"], np.float32)
    b1 = np.asarray(inputs["b1"], np.float32)
    W2 = np.asarray(inputs["W2"], np.float32)
    b2 = np.asarray(inputs["b2"], np.float32)

    bf16 = ml_dtypes.bfloat16
    cstb = _build_cstb(W1, b1, W2)
    cstf = _build_cstf(b2)
    ids_all = np.concatenate(
        [src.astype(np.float32), dst.astype(np.float32)], axis=1
    ).astype(bf16)  # [B, 2S]

    in_maps = []
    for c in range(NCORES):
        sl = slice(c * R, (c + 1) * R)
        in_maps.append({"idsb": ids_all[sl], "cstb": cstb, "cstf": cstf})

    nc = get_module()
    import os

    trace = bool(int(os.environ.get("KERNEL_TRACE", "0")))
    res = bass_utils.run_bass_kernel_spmd(
        nc, in_maps, core_ids=list(range(NCORES)), trace=trace
    )
    LAST_RESULT = res

    src_feat = np.empty((B, S, D), np.float32)
    dst_feat = np.empty((B, S, D), np.float32)
    for c in range(NCORES):
        o = np.asarray(res.results[c]["out"], dtype=np.float32).reshape(
            D, R, 2, S
        )
        sl = slice(c * R, (c + 1) * R)
        src_feat[sl] = o[:, :, 0, :].transpose(1, 2, 0)
        dst_feat[sl] = o[:, :, 1, :].transpose(1, 2, 0)
    return src_feat, dst_feat
